# revision 6
# baseline (speedup 1.0000x reference)
"""2-layer GCN (GCNConv without normalization) as a Bass/Trainium2 SPMD kernel on 8 NeuronCores.

Strategy (graph/data parallel, node sharding):
  - Nodes are sorted by in-degree and dealt round-robin to the 8 cores, so
    every core owns ~E/8 edges and windows of 128 nodes have near-uniform
    max in-degree (minimal gather padding).
  - Layer algebra: segment_sum commutes with the linear maps, so both layers
    aggregate 16-wide tables:
        L1: h1 = x @ W1 (16 wide); agg1 = scatter_add(w * h1[src]);
            a1 = relu(agg1 + b1)
        L2: agg2 = scatter_add(w * a1[src]); out = log_softmax(agg2 @ W2 + b2)
  - Each core computes h1 for its shard, AllGathers the full table, then
    gathers its edges' source rows with one big indirect DMA per chunk,
    multiplies by edge weights (DVE) and segment-reduces per 128-node window
    (strided tensor_reduce). Same for layer 2 with the a1 table.
  - Gather padding: each destination window of 128 nodes (one per partition)
    is padded to the window's max degree; pad slots use table row 0 with
    weight 0.

The harness calls kernel(**inputs) with full inputs; sharding happens here.
"""

import os
import sys
import time

import numpy as np

sys.path.insert(0, "/opt/trn_rl_repo")

NCORES = 8
P = 128
SLOT_BUDGET = 512  # max gather slots per chunk (x64B/partition for 16-wide rows)

# stash for test harness introspection (exec time, etc.)
LAST = {}


def _host_prep(x, edge_index, edge_weight):
    """Build the sharded/padded data layout. Returns (cfg, per_core_arrays)."""
    N, Fin = x.shape
    E = edge_index.shape[1]
    src = np.ascontiguousarray(edge_index[0]).astype(np.int64)
    dst = np.ascontiguousarray(edge_index[1]).astype(np.int64)

    deg = np.bincount(dst, minlength=N)
    order = np.argsort(-deg, kind="stable")  # node ids, highest in-degree first
    rank_of = np.empty(N, np.int64)
    rank_of[order] = np.arange(N)
    core_of = rank_of % NCORES
    lrank = rank_of // NCORES

    NLOC = -(-N // NCORES)          # nodes per core (assumes N % NCORES == 0 for exactness)
    W = -(-NLOC // P)               # windows per core
    NPAD = W * P
    w_of = lrank // P
    p_of = lrank % P
    # gather-table row of each node (same layout for the h1 and a1 tables)
    tbl = (core_of * NPAD + p_of * W + w_of).astype(np.int32)

    # per-window max degree K[w]: window w covers global degree-ranks [w*1024, ...)
    K = np.zeros(W, np.int64)
    for w in range(W):
        lo = w * P * NCORES
        hi = min((w + 1) * P * NCORES, N)
        K[w] = deg[order[lo:hi]].max() if hi > lo else 0
    K = np.maximum(K, 1)  # keep windows non-empty so every reduce is well-formed
    OFF = np.concatenate([[0], np.cumsum(K)]).astype(np.int64)
    S_total = int(OFF[-1])

    # chunk windows so each chunk's slot count fits the SBUF budget
    chunks = []
    w0, s0 = 0, 0
    for w in range(W):
        if s0 + K[w] > SLOT_BUDGET and w > w0:
            chunks.append((w0, w, int(OFF[w0]), int(s0)))
            w0, s0 = w, 0
        s0 += K[w]
    chunks.append((w0, W, int(OFF[w0]), int(s0)))

    # edge slots: sort edges by destination node id; j-th edge of node d goes
    # to (core_of[d], p_of[d], OFF[w_of[d]] + j)
    e_sort = np.argsort(dst, kind="stable")
    ds = dst[e_sort]
    ss = src[e_sort]
    ws = np.ascontiguousarray(edge_weight)[e_sort].astype(np.float32)
    starts = np.concatenate([[0], np.cumsum(deg)])
    j = np.arange(E) - starts[ds]
    col = OFF[w_of[ds]] + j

    idx_arr = np.zeros((NCORES, P, S_total), np.int32)
    wts_arr = np.zeros((NCORES, P, S_total), np.float32)
    idx_arr[core_of[ds], p_of[ds], col] = tbl[ss]
    wts_arr[core_of[ds], p_of[ds], col] = ws

    # x^T shards in slot order: core k, slot l (= w*128+p) -> node order[l*8+k]
    node_of = order[: NLOC * NCORES].reshape(NLOC, NCORES)
    xts = np.zeros((NCORES, Fin, NPAD), np.float32)
    for k in range(NCORES):
        xts[k, :, :NLOC] = x[node_of[:, k]].T

    # output reshuffle: result row p*W+w of core k -> node node_of[w*128+p, k]
    l = np.arange(NLOC)
    out_rows = (l % P) * W + l // P  # row in kernel output for local slot l

    cfg = dict(
        N=N, Fin=Fin, E=E, NLOC=NLOC, W=W, NPAD=NPAD,
        K=K.tolist(), OFF=OFF.tolist(), S_total=S_total, chunks=chunks,
    )
    return cfg, idx_arr, wts_arr, xts, node_of, out_rows


def _build(nc, cfg, H, C):
    """Emit the Bass/Tile program (identical on all cores).

    GCN_STAGE env limits how much of the pipeline is emitted (debug bisect):
      h1 < ag1 < g1 < l1 < ag2 < l2 < full (default)
    """
    import concourse.mybir as mybir
    import concourse.tile as tile
    from concourse import bass
    from concourse.masks import make_identity

    Fin, W, NPAD, S_total = cfg["Fin"], cfg["W"], cfg["NPAD"], cfg["S_total"]
    K, OFF, chunks = cfg["K"], cfg["OFF"], cfg["chunks"]
    KB = Fin // P  # K-blocks for the x @ W1 matmul
    f32 = mybir.dt.float32

    xT = nc.dram_tensor("xT", [Fin, NPAD], f32, kind="ExternalInput")
    W1 = nc.dram_tensor("W1", [Fin, H], f32, kind="ExternalInput")
    W2 = nc.dram_tensor("W2", [H, C], f32, kind="ExternalInput")
    b1b = nc.dram_tensor("b1b", [P, H], f32, kind="ExternalInput")
    b2b = nc.dram_tensor("b2b", [P, C], f32, kind="ExternalInput")
    idx = nc.dram_tensor("idx", [P, S_total], mybir.dt.int32, kind="ExternalInput")
    wts = nc.dram_tensor("wts", [P, S_total], f32, kind="ExternalInput")
    out = nc.dram_tensor("out", [NPAD, C], f32, kind="ExternalOutput")

    h1_shard = nc.dram_tensor("h1_shard", [NPAD, H], f32)
    h1_full = nc.dram_tensor("h1_full", [NCORES * NPAD, H], f32, addr_space="Shared")
    a1_shard = nc.dram_tensor("a1_shard", [NPAD, H], f32)
    a1_full = nc.dram_tensor("a1_full", [NCORES * NPAD, H], f32, addr_space="Shared")

    XTW = 8  # windows per xT load chunk
    STAGE = os.environ.get("GCN_STAGE", "full")
    ORDER = ["h1", "ag1", "g1", "l1", "ag2", "l2", "full"]
    lvl = ORDER.index(STAGE)

    def bcast_mid(ap, n):
        """[P, F] -> [P, n, F] with a step-0 middle dim."""
        return bass.AP(ap.tensor, ap.offset, [list(ap.ap[0]), [0, n], list(ap.ap[1])])

    with tile.TileContext(nc) as tc:
        with (
            tc.tile_pool(name="const", bufs=1) as constp,
            tc.tile_pool(name="persist", bufs=1) as persist,
            tc.tile_pool(name="xt", bufs=2) as xtp,
            tc.tile_pool(name="gath", bufs=3) as gathp,
            tc.tile_pool(name="meta", bufs=3) as metap,
            tc.tile_pool(name="gt", bufs=2) as gtp,
            tc.tile_pool(name="ps_h", bufs=2, space="PSUM") as ps_h,
            tc.tile_pool(name="ps_t", bufs=2, space="PSUM") as ps_t,
            tc.tile_pool(name="ps_o", bufs=2, space="PSUM") as ps_o,
        ):
            # constants
            w1_sb = constp.tile([P, KB * H], f32, tag="w1")
            for kb in range(KB):
                nc.sync.dma_start(out=w1_sb[:, kb * H:(kb + 1) * H],
                                  in_=W1[kb * P:(kb + 1) * P, :])
            w2_sb = constp.tile([H, C], f32, tag="w2")
            nc.sync.dma_start(out=w2_sb[:, :], in_=W2[:, :])
            b1_sb = constp.tile([P, H], f32, tag="b1")
            nc.sync.dma_start(out=b1_sb[:, :], in_=b1b[:, :])
            b2_sb = constp.tile([P, C], f32, tag="b2")
            nc.sync.dma_start(out=b2_sb[:, :], in_=b2b[:, :])
            ident = constp.tile([P, P], f32, tag="ident")
            make_identity(nc, ident[:])

            idx_all = persist.tile([P, S_total], mybir.dt.int32, tag="idxall")
            nc.sync.dma_start(out=idx_all[:, :], in_=idx[:, :])
            wts_all = persist.tile([P, S_total], f32, tag="wtsall")
            nc.sync.dma_start(out=wts_all[:, :], in_=wts[:, :])

            h1_sb = persist.tile([P, W * H], f32, tag="h1")
            agg_sb = persist.tile([P, W * H], f32, tag="agg")
            o_sb = persist.tile([P, W * C], f32, tag="o")
            e_sb = persist.tile([P, W * C], f32, tag="e")
            red_sb = persist.tile([P, 2 * W], f32, tag="red")

            # ---- Phase 1: h1 = x @ W1, per 128-node window ----
            for wc in range(0, W, XTW):
                nw = min(XTW, W - wc)
                xt_sb = xtp.tile([P, KB, XTW * P], f32, tag="xt")
                for kb in range(KB):
                    nc.sync.dma_start(
                        out=xt_sb[:, kb, : nw * P],
                        in_=xT[kb * P:(kb + 1) * P, wc * P:(wc + nw) * P],
                    )
                for w in range(wc, wc + nw):
                    ph = ps_h.tile([P, H], f32, tag="ph")
                    for kb in range(KB):
                        nc.tensor.matmul(
                            out=ph[:, :],
                            lhsT=xt_sb[:, kb, (w - wc) * P:(w - wc + 1) * P],
                            rhs=w1_sb[:, kb * H:(kb + 1) * H],
                            start=(kb == 0),
                            stop=(kb == KB - 1),
                        )
                    nc.scalar.copy(out=h1_sb[:, w * H:(w + 1) * H], in_=ph[:, :])

            # write shard (row = p*W + w, contiguous per partition) and AllGather
            nc.sync.dma_start(
                out=h1_shard[:, :].rearrange("(p w) h -> p (w h)", p=P),
                in_=h1_sb[:, :],
            )
            if lvl < 1:
                return
            nc.gpsimd.collective_compute(
                "AllGather",
                mybir.AluOpType.bypass,
                replica_groups=[list(range(NCORES))],
                ins=[h1_shard[:, :]],
                outs=[h1_full[:, :]],
            )

            # ---- Phases 2/4: edge gather + weight + segment-reduce ----
            def edge_layer(table, dst_sb, only_gather=False):
                for (w0, w1, off0, S_c) in chunks:
                    ga = gathp.tile([P, SLOT_BUDGET * H], f32, tag="ga")
                    # HW indirect DMA consumes ONE index per partition per
                    # instruction: gather the chunk column by column.
                    for c in range(S_c):
                        nc.gpsimd.indirect_dma_start(
                            out=ga[:, c * H:(c + 1) * H],
                            out_offset=None,
                            in_=table[:, :],
                            in_offset=bass.IndirectOffsetOnAxis(
                                ap=idx_all[:, off0 + c:off0 + c + 1], axis=0),
                        )
                    if only_gather:
                        continue
                    # msg *= w  (broadcast each weight over H features)
                    ga3 = ga[:, : S_c * H].rearrange("p (s h) -> p s h", h=H)
                    nc.vector.tensor_tensor(
                        out=ga3,
                        in0=ga3,
                        in1=wts_all[:, off0:off0 + S_c].to_broadcast([P, S_c, H]),
                        op=mybir.AluOpType.mult,
                    )
                    for w in range(w0, w1):
                        o = (OFF[w] - off0) * H
                        nc.vector.tensor_reduce(
                            out=dst_sb[:, w * H:(w + 1) * H],
                            in_=ga[:, o: o + K[w] * H].rearrange(
                                "p (s h) -> p h s", h=H),
                            axis=mybir.AxisListType.X,
                            op=mybir.AluOpType.add,
                        )

            if lvl < 2:
                return
            if lvl == 2:
                edge_layer(h1_full, agg_sb, only_gather=True)
                return
            edge_layer(h1_full, agg_sb)

            # ---- Phase 3: a1 = relu(agg1 + b1); share and AllGather ----
            agg3 = agg_sb[:, :].rearrange("p (w h) -> p w h", h=H)
            nc.vector.tensor_tensor(
                out=agg3, in0=agg3, in1=bcast_mid(b1_sb[:, :], W),
                op=mybir.AluOpType.add,
            )
            nc.vector.tensor_scalar_max(out=agg_sb[:, :], in0=agg_sb[:, :], scalar1=0.0)
            nc.sync.dma_start(
                out=a1_shard[:, :].rearrange("(p w) h -> p (w h)", p=P),
                in_=agg_sb[:, :],
            )
            if lvl < 4:
                return
            nc.gpsimd.collective_compute(
                "AllGather",
                mybir.AluOpType.bypass,
                replica_groups=[list(range(NCORES))],
                ins=[a1_shard[:, :]],
                outs=[a1_full[:, :]],
            )

            if lvl < 5:
                return
            edge_layer(a1_full, h1_sb)  # reuse h1_sb as G (L2 aggregate)
            if lvl < 6:
                return

            # ---- Phase 5: out = log_softmax(G @ W2 + b2) ----
            for w in range(W):
                pt = ps_t.tile([H, P], f32, tag="pt")
                nc.tensor.transpose(
                    out=pt[:, :], in_=h1_sb[:, w * H:(w + 1) * H], identity=ident[:]
                )
                gt_sb = gtp.tile([H, P], f32, tag="gt")
                nc.scalar.copy(out=gt_sb[:, :], in_=pt[:, :])
                po = ps_o.tile([P, C], f32, tag="po")
                nc.tensor.matmul(
                    out=po[:, :], lhsT=gt_sb[:, :], rhs=w2_sb[:, :],
                    start=True, stop=True,
                )
                nc.scalar.copy(out=o_sb[:, w * C:(w + 1) * C], in_=po[:, :])

            o3 = o_sb[:, :].rearrange("p (w c) -> p w c", c=C)
            nc.vector.tensor_tensor(
                out=o3, in0=o3,
                in1=bcast_mid(b2_sb[:, :], W),
                op=mybir.AluOpType.add,
            )
            rmax = red_sb[:, 0:W]
            rsum = red_sb[:, W:2 * W]
            nc.vector.tensor_reduce(out=rmax, in_=o3, axis=mybir.AxisListType.X,
                                    op=mybir.AluOpType.max)
            nc.vector.tensor_tensor(out=o3, in0=o3,
                                    in1=rmax.to_broadcast([P, W, C]),
                                    op=mybir.AluOpType.subtract)
            nc.scalar.activation(out=e_sb[:, :], in_=o_sb[:, :],
                                 func=mybir.ActivationFunctionType.Exp)
            nc.vector.tensor_reduce(out=rsum,
                                    in_=e_sb[:, :].rearrange("p (w c) -> p w c", c=C),
                                    axis=mybir.AxisListType.X,
                                    op=mybir.AluOpType.add)
            nc.scalar.activation(out=rsum, in_=rsum,
                                 func=mybir.ActivationFunctionType.Ln)
            nc.vector.tensor_tensor(out=o3, in0=o3,
                                    in1=rsum.to_broadcast([P, W, C]),
                                    op=mybir.AluOpType.subtract)
            nc.sync.dma_start(
                out=out[:, :].rearrange("(p w) c -> p (w c)", p=P),
                in_=o_sb[:, :],
            )
    return None


def kernel(x, edge_index, edge_weight, W1, b1, W2, b2):
    import concourse.bacc as bacc
    from concourse.bass_utils import run_bass_kernel_spmd

    x = np.asarray(x, dtype=np.float32)
    W1 = np.asarray(W1, dtype=np.float32)
    b1 = np.asarray(b1, dtype=np.float32)
    W2 = np.asarray(W2, dtype=np.float32)
    b2 = np.asarray(b2, dtype=np.float32)
    edge_weight = np.asarray(edge_weight, dtype=np.float32)
    edge_index = np.asarray(edge_index)

    N = x.shape[0]
    H = W1.shape[1]
    C = W2.shape[1]

    t0 = time.time()
    cfg, idx_arr, wts_arr, xts, node_of, out_rows = _host_prep(x, edge_index, edge_weight)
    LAST["prep_s"] = time.time() - t0

    t0 = time.time()
    nc = bacc.Bacc("TRN2", target_bir_lowering=False, debug=False, num_devices=NCORES)
    _build(nc, cfg, H, C)
    nc.compile()
    LAST["build_s"] = time.time() - t0

    b1b = np.broadcast_to(b1, (P, H)).copy()
    b2b = np.broadcast_to(b2, (P, C)).copy()
    in_maps = [
        {
            "xT": xts[k],
            "W1": W1, "W2": W2, "b1b": b1b, "b2b": b2b,
            "idx": idx_arr[k], "wts": wts_arr[k],
        }
        for k in range(NCORES)
    ]

    t0 = time.time()
    res = run_bass_kernel_spmd(
        nc, in_maps, core_ids=list(range(NCORES)),
        trace=bool(int(os.environ.get("GCN_TRACE", "0"))),
    )
    LAST["run_s"] = time.time() - t0
    LAST["results"] = res
    LAST["cfg"] = cfg

    outf = np.empty((N, C), np.float32)
    for k in range(NCORES):
        outf[node_of[:, k]] = res.results[k]["out"][out_rows]
    return outf
